# revision 26
# baseline (speedup 1.0000x reference)
"""Trainium2 Bass kernel for nn_KeyRecorder (optimized v6).

Math (reference):
  comp = LN(relu(obs @ W1 + b1)) * g1 + bl1          [B, T, R]
  past = max(comp[:, :-20:10, :], axis=time)          408 strided rows
  gmax = max(cummax(comp[:, -20:, :]), past)          [B, 20, R]
  out  = LN(relu(gmax @ W2 + b2)) * g2 + bl2          [B, 20, D]

Only 428 of the 4096 timesteps per batch element are consumed (408
strided + last 20); the host gathers those rows, pads each batch
element to 448 tokens and ships them transposed (d-major) in fp16
(~0.46 MB per batch element).  Batch is sharded 2-per-core across 8
cores; the two batch elements run as two symmetric, fully independent
pipelines so the first one starts as soon as its own 229 KB lands.

Per batch element (448 token cols = 4 slabs of 112):
  - W1-stationary fp16 matmuls -> psum [64,448]; ACT relu(x+b1) ->
    fp16 [r,t]; per-slab transpose via a [64,65] (identity | ones)
    matmul lands [112 tok, 64 feat + rowsum] in psum; batched LN
    stats on [112,4] tiles (variance via E[x2]-mu^2, rstd via a raw
    Rsqrt activation); fused (x-mu)*rstd applies alternate DVE/ACT;
    transpose back to [r,t] psum.
  - past = reduce_max over the 408 strided cols (split so most of it
    overlaps the last transpose); seeded running max = one
    tensor_tensor_scan (hw prefix scan, initial=past).
  - stage 3: [65,20]x[65,512] matmul (ones row adds b2), ACT relu +
    fused row-sum, square + row-sum (DVE for elem 0, ACT for elem 1),
    fused (x-mu)*rstd apply, DMA out.

Runtime notes: scalar-engine dma_start crashes the exec unit
(NRT_EXEC_UNIT_UNRECOVERABLE) — obs rides the sync queue, weights ride
the gpsimd queue; tensor_tensor_reduce also crashes; GPSIMD cannot
read PSUM (BIR verifier).  A dummy Rsqrt primes the one ACT table
(reciprocal_sqrt_and_small) serving Relu/Square/Identity/Rsqrt.

Affine folds (host side): LN1's g1/bl1 fold into W2/b2 (g1 >= 0
asserted; max/cummax commute with monotone maps); LN2's g2/bl2 are
applied to the gathered output on the host.
"""

import os
import numpy as np

import concourse.bass as bass
import concourse.bacc as bacc
import concourse.mybir as mybir
import concourse.tile as tile
from concourse.bass_utils import run_bass_kernel_spmd

F32 = mybir.dt.float32
F16 = mybir.dt.float16
ALU = mybir.AluOpType
ACT = mybir.ActivationFunctionType
AX = mybir.AxisListType

B, T, D, R = 16, 4096, 512, 64
LOCAL, SR, EPS = 20, 10, 1e-5
N_CORES = 8
BPC = B // N_CORES                   # batch elements per core
NSTR = (T - LOCAL + SR - 1) // SR    # 408 strided past rows
NSEL = NSTR + LOCAL                  # 428 rows consumed per batch elem
GRP = 448                            # per-batch token width (428 padded)
SLAB = 112                           # 4 slabs per batch element
NSLB = GRP // SLAB                   # 4
DC = D // 128                        # 4 contraction chunks
NO = BPC * LOCAL                     # 40 output rows per core
PSPLIT = 3 * SLAB                    # past reduce split point (336)

# packed weight tensor column offsets (fp16).  Piece 1 (cols 0..450)
# holds everything stage 1 needs; piece 2 (cols 450..962) only feeds
# stage 3, so it can trail on the gpsimd queue.
WCOL_ID128 = 256
WCOL_IDP = 384
WCOL_B1 = 449
WCOL_W2 = 450
WPACK = 962

IDX = np.array(list(range(0, T - LOCAL, SR)) + list(range(T - LOCAL, T)))

_cache: dict = {}


def _build_program():
    if "nc" in _cache:
        return _cache["nc"]

    nc = bacc.Bacc("TRN2", target_bir_lowering=False, debug=False,
                   enable_asserts=False)

    obs_d = [nc.dram_tensor(f"obs{h}", [128, DC, GRP], F16,
                            kind="ExternalInput") for h in range(BPC)]
    wp_d = nc.dram_tensor("wpack", [128, WPACK], F16, kind="ExternalInput")
    out_d = nc.dram_tensor("out", [NO, D], F32, kind="ExternalOutput")

    inv_r = 1.0 / R
    inv_d = 1.0 / D

    with tile.TileContext(nc) as tc:
        with (
            tc.tile_pool(name="const", bufs=1) as cpool,
            tc.tile_pool(name="pg", bufs=2, space=bass.MemorySpace.PSUM) as ppg,
            tc.tile_pool(name="xr", bufs=2, space=bass.MemorySpace.PSUM) as pxr,
            tc.tile_pool(name="ct", bufs=2, space=bass.MemorySpace.PSUM) as pct,
            tc.tile_pool(name="o3", bufs=2, space=bass.MemorySpace.PSUM) as po3,
        ):
            # ---------- SBUF tiles ----------
            wp = cpool.tile([128, WPACK], F16)
            dmy = cpool.tile([1, 1], F32)
            dmyo = cpool.tile([1, 1], F32)
            SCW = NSEL - PSPLIT                       # 92-col scan window
            neginf = cpool.tile([R, SCW], F16)
            eps_t = cpool.tile([128, 1], F32)
            scr = [cpool.tile([R + 1, SCW], F16, name=f"scr{h}")
                   for h in range(BPC)]
            obs_sb, s1, st3 = [], [], []
            for h in range(BPC):
                obs_sb.append(cpool.tile([128, DC, GRP], F16,
                                         name=f"obs_sb{h}"))
                sh1 = dict(xrT=([R, GRP], F16), sq=([SLAB, NSLB, R], F16),
                           y=([SLAB, NSLB, R], F16), mu=([SLAB, NSLB], F32),
                           ssq=([SLAB, NSLB], F32), mu2=([SLAB, NSLB], F32),
                           var=([SLAB, NSLB], F32), rstd=([SLAB, NSLB], F32),
                           negmu=([SLAB, NSLB], F32), nmr=([SLAB, NSLB], F32),
                           p0=([R, 1], F32))
                s1.append({k: cpool.tile(sh, dt, name=f"s1_{k}{h}")
                           for k, (sh, dt) in sh1.items()})
                sh3 = dict(xr=([LOCAL, D], F16), sq=([LOCAL, D], F16),
                           rsum=([LOCAL, 1], F32), ssq=([LOCAL, 1], F32),
                           mu=([LOCAL, 1], F32), mu2=([LOCAL, 1], F32),
                           var=([LOCAL, 1], F32), rstd=([LOCAL, 1], F32),
                           out=([LOCAL, D], F32))
                st3.append({k: cpool.tile(sh, dt, name=f"s3_{k}{h}")
                            for k, (sh, dt) in sh3.items()})

            w1c = lambda c: wp[:, 64 * c:64 * (c + 1)]
            id112 = wp[0:SLAB, WCOL_ID128:WCOL_ID128 + SLAB]
            idp = wp[0:R, WCOL_IDP:WCOL_IDP + R + 1]
            w2aug = wp[0:R + 1, WCOL_W2:WCOL_W2 + D]
            b1col = wp[0:R, WCOL_B1:WCOL_B1 + 1]

            # ---------- constants ----------
            nc.gpsimd.memset(dmy[:], 1.0)
            nc.gpsimd.memset(neginf[:], -60000.0)
            nc.gpsimd.memset(scr[0][R:R + 1, :], 1.0)
            nc.gpsimd.memset(scr[1][R:R + 1, :], 1.0)
            nc.gpsimd.memset(eps_t[:], EPS)

            # ---------- DMA in: obs on sync, weights on gpsimd ----------
            q2 = nc.gpsimd if os.environ.get("KV_GPS_DMA", "1") != "0" \
                else nc.sync
            nc.sync.dma_start(obs_sb[0][:], obs_d[0][:])
            q2.dma_start(obs_sb[1][:], obs_d[1][:])
            nc.sync.dma_start(wp[:, 0:WCOL_W2], wp_d[:, 0:WCOL_W2])
            q2.dma_start(wp[:, WCOL_W2:WPACK], wp_d[:, WCOL_W2:WPACK])

            # raw Rsqrt activation: the bass wrapper refuses Rsqrt on
            # accuracy grounds; ~1e-3 relative is fine here and it fuses
            # sqrt+reciprocal into one op.
            def rsqrt_act(out, in_, bias):
                eng = nc.scalar
                ins_ = [eng.lower_ap(in_), eng.lower_ap(bias),
                        mybir.ImmediateValue(dtype=mybir.dt.float32,
                                             value=1.0),
                        mybir.ImmediateValue(dtype=mybir.dt.float32,
                                             value=0.0)]
                return eng.add_instruction(mybir.InstActivation(
                    name=eng.bass.get_next_instruction_name(),
                    func=ACT.Rsqrt, ins=ins_, outs=[eng.lower_ap(out)]))

            # primes the single ACT table (reciprocal_sqrt_and_small)
            rsqrt_act(dmyo[:], dmy[:], eps_t[0:1, :])

            def pipeline(h, ps3):
                t, u = s1[h], st3[h]
                obs = obs_sb[h]

                # stage 1: z = obs @ W1 + b1, relu -> [r, t] fp16
                pg = ppg.tile([R, GRP], F32, tag="pg")
                for c in range(DC):
                    nc.tensor.matmul(pg[:], w1c(c), obs[:, c, :],
                                     start=(c == 0), stop=(c == DC - 1))
                nc.scalar.activation(t["xrT"][:], pg[:], ACT.Relu,
                                     bias=b1col, scale=1.0)

                # transpose slabs with fused row-sums (col 64)
                xrp = pxr.tile([128, NSLB, R + 1], F32, tag="xr")
                for j in range(NSLB):
                    nc.tensor.matmul(xrp[0:SLAB, j, :],
                                     t["xrT"][:, SLAB * j:SLAB * (j + 1)],
                                     idp, start=True, stop=True)

                # batched LN stats on [112, 4]
                nc.vector.tensor_scalar_mul(t["mu"][:], xrp[0:SLAB, :, R],
                                            inv_r)
                nc.scalar.activation(t["sq"][:], xrp[0:SLAB, :, 0:R],
                                     ACT.Square)
                nc.gpsimd.tensor_tensor(t["mu2"][:], t["mu"][:], t["mu"][:],
                                        op=ALU.mult)
                nc.gpsimd.tensor_scalar_mul(t["negmu"][:], t["mu"][:], -1.0)
                nc.vector.reduce_sum(t["ssq"][:], t["sq"][:], axis=AX.X)
                nc.vector.scalar_tensor_tensor(t["var"][:], t["ssq"][:],
                                               inv_r, t["mu2"][:],
                                               ALU.mult, ALU.subtract)
                rsqrt_act(t["rstd"][:], t["var"][:], eps_t[0:SLAB, :])
                nc.gpsimd.tensor_tensor(t["nmr"][:], t["negmu"][:],
                                        t["rstd"][:], op=ALU.mult)

                # apply LN + transpose back to [r, t] psum
                ct = pct.tile([R, GRP], F32, tag="ct")
                for j in range(NSLB):
                    if j % 2 == 1:
                        nc.scalar.activation(t["y"][:, j, :],
                                             xrp[0:SLAB, j, 0:R],
                                             ACT.Identity,
                                             bias=t["nmr"][:, j:j + 1],
                                             scale=t["rstd"][:, j:j + 1])
                    else:
                        nc.vector.tensor_scalar(t["y"][:, j, :],
                                                xrp[0:SLAB, j, 0:R],
                                                t["mu"][:, j:j + 1],
                                                t["rstd"][:, j:j + 1],
                                                ALU.subtract, ALU.mult)
                    nc.tensor.matmul(ct[:, SLAB * j:SLAB * (j + 1)],
                                     t["y"][:, j, :], id112,
                                     start=True, stop=True)
                    if j == 2:
                        # most of the past max overlaps the last slab
                        nc.vector.reduce_max(t["p0"][:], ct[:, 0:PSPLIT],
                                             axis=AX.X)

                # running max through the remaining strided cols reaches
                # the full past by col 408; the last 20 outputs are gmax
                nc.vector.tensor_tensor_scan(
                    scr[h][0:R, :], ct[:, PSPLIT:NSEL], neginf[:],
                    t["p0"][:], ALU.max, ALU.max)

                # stage 3: out = LN(relu(gmax @ W2' + b2'))
                nc.tensor.matmul(ps3[0:LOCAL, :],
                                 scr[h][:, SCW - LOCAL:SCW],
                                 w2aug, start=True, stop=True)
                if h == 0:
                    # b0's whole relu/square path rides DVE while ACT
                    # finishes b1's stage-1 work
                    nc.vector.tensor_scalar(u["xr"][:], ps3[0:LOCAL, :],
                                            0.0, 0.0, ALU.max, ALU.add,
                                            accum_out=u["rsum"][:])
                    nc.vector.tensor_tensor(u["sq"][:], u["xr"][:],
                                            u["xr"][:], op=ALU.mult)
                    nc.vector.reduce_sum(u["ssq"][:], u["sq"][:], axis=AX.X)
                else:
                    nc.scalar.activation(u["xr"][:], ps3[0:LOCAL, :],
                                         ACT.Relu, accum_out=u["rsum"][:])
                    nc.scalar.activation(u["sq"][:], u["xr"][:], ACT.Square,
                                         accum_out=u["ssq"][:])
                nc.vector.tensor_scalar_mul(u["mu"][:], u["rsum"][:], inv_d)
                nc.vector.tensor_tensor(u["mu2"][:], u["mu"][:], u["mu"][:],
                                        op=ALU.mult)
                nc.vector.scalar_tensor_tensor(u["var"][:], u["ssq"][:],
                                               inv_d, u["mu2"][:],
                                               ALU.mult, ALU.subtract)
                rsqrt_act(u["rstd"][:], u["var"][:], eps_t[0:LOCAL, :])
                nc.vector.tensor_scalar(u["out"][:], u["xr"][:], u["mu"][:],
                                        u["rstd"][:], ALU.subtract, ALU.mult)
                oq = q2 if h == 0 else nc.sync
                oq.dma_start(out_d[LOCAL * h:LOCAL * (h + 1), :],
                             u["out"][:])

            ps3a = po3.tile([LOCAL, D], F32, tag="o3")
            ps3b = po3.tile([LOCAL, D], F32, tag="o3")
            pipeline(0, ps3a)
            pipeline(1, ps3b)

    nc.compile()
    _cache["nc"] = nc
    return nc


def _host_inputs(obs, W1, b1, ln1_g, ln1_b, W2, b2):
    obs = np.ascontiguousarray(np.asarray(obs, dtype=np.float32))
    W1 = np.asarray(W1, np.float32)
    b1 = np.asarray(b1, np.float32)
    ln1_g = np.asarray(ln1_g, np.float32)
    ln1_b = np.asarray(ln1_b, np.float32)
    W2 = np.asarray(W2, np.float32)
    b2 = np.asarray(b2, np.float32)

    # folding LN1's affine past the max/cummax requires monotonicity
    assert np.all(ln1_g >= 0), "ln1_g must be >= 0 for the affine fold"

    wpack = np.zeros((128, WPACK), np.float16)
    wpack[:, 0:256] = W1.reshape(DC, 128, R).transpose(1, 0, 2).reshape(
        128, 256).astype(np.float16)
    wpack[:, WCOL_ID128:WCOL_ID128 + 128] = np.eye(128, dtype=np.float16)
    wpack[0:R, WCOL_IDP:WCOL_IDP + R] = np.eye(R, dtype=np.float16)
    wpack[0:R, WCOL_IDP + R] = 1.0
    wpack[0:R, WCOL_B1] = b1.astype(np.float16)
    wpack[0:R + 1, WCOL_W2:WCOL_W2 + D] = np.concatenate(
        [ln1_g[:, None] * W2, (b2 + ln1_b @ W2)[None, :]],
        axis=0).astype(np.float16)

    shared = {"wpack": wpack}
    in_maps = []
    for c in range(N_CORES):
        sel = obs[BPC * c:BPC * (c + 1)][:, IDX, :]        # [BPC, 428, 512]
        grp = np.zeros((BPC, GRP, D), np.float32)
        grp[:, :NSEL] = sel
        m = {}
        for h in range(BPC):
            obsT = grp[h].T                                 # [512, 448]
            obsf = obsT.reshape(DC, 128, GRP).transpose(1, 0, 2)
            m[f"obs{h}"] = np.ascontiguousarray(obsf.astype(np.float16))
        in_maps.append({**m, **shared})
    return in_maps


def _install_ntff_shim():
    """The agent image's antenv lacks axon_hooks; synthesize it so
    trace=True can reach the libaxon NTFF profiler (test-time only)."""
    import sys
    import types
    if "antenv.axon_hooks" in sys.modules:
        return True
    try:
        import antenv
        from trn_agent_boot.trn_boot import _ntff_profile_via_ctypes
    except ImportError:
        return False
    so_path = "/opt/axon/libaxon_pjrt.so"
    if not os.path.exists(so_path):
        return False
    hook = _ntff_profile_via_ctypes(so_path)
    mod = types.ModuleType("antenv.axon_hooks")
    mod._hook = hook
    mod.set_axon_ntff_profile_hook = lambda h: setattr(mod, "_hook", h)
    mod.get_axon_ntff_profile_hook = lambda: mod._hook
    sys.modules["antenv.axon_hooks"] = mod
    antenv.axon_hooks = mod
    return hook is not None


def kernel(obs_frames, W1, b1, ln1_g, ln1_b, W2, b2, ln2_g, ln2_b):
    nc = _build_program()
    in_maps = _host_inputs(obs_frames, W1, b1, ln1_g, ln1_b, W2, b2)
    trace = bool(os.environ.get("BASS_TRACE"))
    if trace:
        trace = _install_ntff_shim()
        import concourse.bass_utils as _bu
        _bu.upload_artifacts = lambda tmpdir: f"local://{tmpdir}"
    res = run_bass_kernel_spmd(nc, in_maps, core_ids=list(range(N_CORES)),
                               trace=trace)
    _cache["last_result"] = res
    out = np.stack([res.results[c]["out"].reshape(BPC, LOCAL, D)
                    for c in range(N_CORES)])
    out = out.reshape(B, LOCAL, D)

    # LN2's affine applied host-side (identity for the given inputs)
    g2 = np.asarray(ln2_g, np.float32)
    b2l = np.asarray(ln2_b, np.float32)
    if not (np.all(g2 == 1.0) and np.all(b2l == 0.0)):
        out = out * g2 + b2l
    return np.ascontiguousarray(out.astype(np.float32))


# revision 27
# speedup vs baseline: 1.0730x; 1.0730x over previous
"""Trainium2 Bass kernel for nn_KeyRecorder (optimized v6).

Math (reference):
  comp = LN(relu(obs @ W1 + b1)) * g1 + bl1          [B, T, R]
  past = max(comp[:, :-20:10, :], axis=time)          408 strided rows
  gmax = max(cummax(comp[:, -20:, :]), past)          [B, 20, R]
  out  = LN(relu(gmax @ W2 + b2)) * g2 + bl2          [B, 20, D]

Only 428 of the 4096 timesteps per batch element are consumed (408
strided + last 20); the host gathers those rows, pads each batch
element to 448 tokens and ships them transposed (d-major) in fp16
(~0.46 MB per batch element).  Batch is sharded 2-per-core across 8
cores; the two batch elements run as two symmetric, fully independent
pipelines so the first one starts as soon as its own 229 KB lands.

Per batch element (448 token cols = 4 slabs of 112):
  - W1-stationary fp16 matmuls -> psum [64,448]; ACT relu(x+b1) ->
    fp16 [r,t]; per-slab transpose via a [64,65] (identity | ones)
    matmul lands [112 tok, 64 feat + rowsum] in psum; batched LN
    stats on [112,4] tiles (variance via E[x2]-mu^2, rstd via a raw
    Rsqrt activation); fused (x-mu)*rstd applies alternate DVE/ACT;
    transpose back to [r,t] psum.
  - past = reduce_max over the 408 strided cols (split so most of it
    overlaps the last transpose); seeded running max = one
    tensor_tensor_scan (hw prefix scan, initial=past).
  - stage 3: [65,20]x[65,512] matmul (ones row adds b2), ACT relu +
    fused row-sum, square + row-sum (DVE for elem 0, ACT for elem 1),
    fused (x-mu)*rstd apply, DMA out.

Runtime notes: scalar-engine dma_start crashes the exec unit
(NRT_EXEC_UNIT_UNRECOVERABLE) — obs rides the sync queue, weights ride
the gpsimd queue; tensor_tensor_reduce also crashes; GPSIMD cannot
read PSUM (BIR verifier).  A dummy Rsqrt primes the one ACT table
(reciprocal_sqrt_and_small) serving Relu/Square/Identity/Rsqrt.

Affine folds (host side): LN1's g1/bl1 fold into W2/b2 (g1 >= 0
asserted; max/cummax commute with monotone maps); LN2's g2/bl2 are
applied to the gathered output on the host.
"""

import os
import numpy as np

import concourse.bass as bass
import concourse.bacc as bacc
import concourse.mybir as mybir
import concourse.tile as tile
from concourse.bass_utils import run_bass_kernel_spmd

F32 = mybir.dt.float32
F16 = mybir.dt.float16
ALU = mybir.AluOpType
ACT = mybir.ActivationFunctionType
AX = mybir.AxisListType

B, T, D, R = 16, 4096, 512, 64
LOCAL, SR, EPS = 20, 10, 1e-5
N_CORES = 8
BPC = B // N_CORES                   # batch elements per core
NSTR = (T - LOCAL + SR - 1) // SR    # 408 strided past rows
NSEL = NSTR + LOCAL                  # 428 rows consumed per batch elem
GRP = 448                            # per-batch token width (428 padded)
SLAB = 112                           # 4 slabs per batch element
NSLB = GRP // SLAB                   # 4
DC = D // 128                        # 4 contraction chunks
NO = BPC * LOCAL                     # 40 output rows per core
PSPLIT = 3 * SLAB                    # past reduce split point (336)

# packed weight tensor column offsets (fp16).  Piece 1 (cols 0..450)
# holds everything stage 1 needs; piece 2 (cols 450..962) only feeds
# stage 3, so it can trail on the gpsimd queue.
WCOL_ID128 = 256
WCOL_IDP = 384
WCOL_B1 = 449
WCOL_W2 = 450
WPACK = 962

IDX = np.array(list(range(0, T - LOCAL, SR)) + list(range(T - LOCAL, T)))

_cache: dict = {}


def _build_program():
    if "nc" in _cache:
        return _cache["nc"]

    nc = bacc.Bacc("TRN2", target_bir_lowering=False, debug=False,
                   enable_asserts=False)

    obs_d = [nc.dram_tensor(f"obs{h}", [128, DC, GRP], F16,
                            kind="ExternalInput") for h in range(BPC)]
    wp_d = nc.dram_tensor("wpack", [128, WPACK], F16, kind="ExternalInput")
    out_d = nc.dram_tensor("out", [NO, D], F32, kind="ExternalOutput")

    inv_r = 1.0 / R
    inv_d = 1.0 / D

    with tile.TileContext(nc) as tc:
        with (
            tc.tile_pool(name="const", bufs=1) as cpool,
            tc.tile_pool(name="pg", bufs=2, space=bass.MemorySpace.PSUM) as ppg,
            tc.tile_pool(name="xr", bufs=2, space=bass.MemorySpace.PSUM) as pxr,
            tc.tile_pool(name="ct", bufs=2, space=bass.MemorySpace.PSUM) as pct,
            tc.tile_pool(name="o3", bufs=2, space=bass.MemorySpace.PSUM) as po3,
        ):
            # ---------- SBUF tiles ----------
            wp = cpool.tile([128, WPACK], F16)
            dmy = cpool.tile([1, 1], F32)
            dmyo = cpool.tile([1, 1], F32)
            SCW = NSEL - PSPLIT                       # 92-col scan window
            neginf = cpool.tile([R, SCW], F16)
            eps_t = cpool.tile([128, 1], F32)
            scr = [cpool.tile([R + 1, SCW], F16, name=f"scr{h}")
                   for h in range(BPC)]
            obs_sb, s1, st3 = [], [], []
            for h in range(BPC):
                obs_sb.append(cpool.tile([128, DC, GRP], F16,
                                         name=f"obs_sb{h}"))
                sh1 = dict(xrT=([R, GRP], F16), sq=([SLAB, NSLB, R], F16),
                           y=([SLAB, NSLB, R], F16), mu=([SLAB, NSLB], F32),
                           ssq=([SLAB, NSLB], F32), mu2=([SLAB, NSLB], F32),
                           var=([SLAB, NSLB], F32), rstd=([SLAB, NSLB], F32),
                           negmu=([SLAB, NSLB], F32), nmr=([SLAB, NSLB], F32),
                           p0=([R, 1], F32))
                s1.append({k: cpool.tile(sh, dt, name=f"s1_{k}{h}")
                           for k, (sh, dt) in sh1.items()})
                sh3 = dict(xr=([LOCAL, D], F16), sq=([LOCAL, D], F16),
                           rsum=([LOCAL, 1], F32), ssq=([LOCAL, 1], F32),
                           mu=([LOCAL, 1], F32), mu2=([LOCAL, 1], F32),
                           var=([LOCAL, 1], F32), rstd=([LOCAL, 1], F32),
                           out=([LOCAL, D], F32))
                st3.append({k: cpool.tile(sh, dt, name=f"s3_{k}{h}")
                            for k, (sh, dt) in sh3.items()})

            w1c = lambda c: wp[:, 64 * c:64 * (c + 1)]
            id112 = wp[0:SLAB, WCOL_ID128:WCOL_ID128 + SLAB]
            idp = wp[0:R, WCOL_IDP:WCOL_IDP + R + 1]
            w2aug = wp[0:R + 1, WCOL_W2:WCOL_W2 + D]
            b1col = wp[0:R, WCOL_B1:WCOL_B1 + 1]

            # ---------- constants ----------
            nc.gpsimd.memset(dmy[:], 1.0)
            nc.gpsimd.memset(neginf[:], -60000.0)
            nc.gpsimd.memset(scr[0][R:R + 1, :], 1.0)
            nc.gpsimd.memset(scr[1][R:R + 1, :], 1.0)
            nc.gpsimd.memset(eps_t[:], EPS)

            # ---------- DMA in: obs on sync, weights on gpsimd ----------
            q2 = nc.gpsimd if os.environ.get("KV_GPS_DMA", "1") != "0" \
                else nc.sync
            nc.sync.dma_start(obs_sb[0][:], obs_d[0][:])
            nc.sync.dma_start(wp[:, 0:WCOL_W2], wp_d[:, 0:WCOL_W2])
            nc.sync.dma_start(obs_sb[1][:], obs_d[1][:])
            q2.dma_start(wp[:, WCOL_W2:WPACK], wp_d[:, WCOL_W2:WPACK])

            # raw Rsqrt activation: the bass wrapper refuses Rsqrt on
            # accuracy grounds; ~1e-3 relative is fine here and it fuses
            # sqrt+reciprocal into one op.
            def rsqrt_act(out, in_, bias):
                eng = nc.scalar
                ins_ = [eng.lower_ap(in_), eng.lower_ap(bias),
                        mybir.ImmediateValue(dtype=mybir.dt.float32,
                                             value=1.0),
                        mybir.ImmediateValue(dtype=mybir.dt.float32,
                                             value=0.0)]
                return eng.add_instruction(mybir.InstActivation(
                    name=eng.bass.get_next_instruction_name(),
                    func=ACT.Rsqrt, ins=ins_, outs=[eng.lower_ap(out)]))

            # primes the single ACT table (reciprocal_sqrt_and_small)
            rsqrt_act(dmyo[:], dmy[:], eps_t[0:1, :])

            def pipeline(h, ps3):
                t, u = s1[h], st3[h]
                obs = obs_sb[h]

                # stage 1: z = obs @ W1 + b1, relu -> [r, t] fp16
                pg = ppg.tile([R, GRP], F32, tag="pg")
                for c in range(DC):
                    nc.tensor.matmul(pg[:], w1c(c), obs[:, c, :],
                                     start=(c == 0), stop=(c == DC - 1))
                nc.scalar.activation(t["xrT"][:], pg[:], ACT.Relu,
                                     bias=b1col, scale=1.0)

                # transpose slabs with fused row-sums (col 64)
                xrp = pxr.tile([128, NSLB, R + 1], F32, tag="xr")
                for j in range(NSLB):
                    nc.tensor.matmul(xrp[0:SLAB, j, :],
                                     t["xrT"][:, SLAB * j:SLAB * (j + 1)],
                                     idp, start=True, stop=True)

                # batched LN stats on [112, 4]
                nc.vector.tensor_scalar_mul(t["mu"][:], xrp[0:SLAB, :, R],
                                            inv_r)
                nc.scalar.activation(t["sq"][:], xrp[0:SLAB, :, 0:R],
                                     ACT.Square)
                nc.gpsimd.tensor_tensor(t["mu2"][:], t["mu"][:], t["mu"][:],
                                        op=ALU.mult)
                nc.gpsimd.tensor_scalar_mul(t["negmu"][:], t["mu"][:], -1.0)
                nc.vector.reduce_sum(t["ssq"][:], t["sq"][:], axis=AX.X)
                nc.vector.scalar_tensor_tensor(t["var"][:], t["ssq"][:],
                                               inv_r, t["mu2"][:],
                                               ALU.mult, ALU.subtract)
                rsqrt_act(t["rstd"][:], t["var"][:], eps_t[0:SLAB, :])
                nc.gpsimd.tensor_tensor(t["nmr"][:], t["negmu"][:],
                                        t["rstd"][:], op=ALU.mult)

                # apply LN + transpose back to [r, t] psum
                ct = pct.tile([R, GRP], F32, tag="ct")
                for j in range(NSLB):
                    if j % 2 == 1:
                        nc.scalar.activation(t["y"][:, j, :],
                                             xrp[0:SLAB, j, 0:R],
                                             ACT.Identity,
                                             bias=t["nmr"][:, j:j + 1],
                                             scale=t["rstd"][:, j:j + 1])
                    else:
                        nc.vector.tensor_scalar(t["y"][:, j, :],
                                                xrp[0:SLAB, j, 0:R],
                                                t["mu"][:, j:j + 1],
                                                t["rstd"][:, j:j + 1],
                                                ALU.subtract, ALU.mult)
                    nc.tensor.matmul(ct[:, SLAB * j:SLAB * (j + 1)],
                                     t["y"][:, j, :], id112,
                                     start=True, stop=True)
                    if j == 2:
                        # most of the past max overlaps the last slab
                        nc.vector.reduce_max(t["p0"][:], ct[:, 0:PSPLIT],
                                             axis=AX.X)

                # running max through the remaining strided cols reaches
                # the full past by col 408; the last 20 outputs are gmax
                nc.vector.tensor_tensor_scan(
                    scr[h][0:R, :], ct[:, PSPLIT:NSEL], neginf[:],
                    t["p0"][:], ALU.max, ALU.max)

                # stage 3: out = LN(relu(gmax @ W2' + b2'))
                nc.tensor.matmul(ps3[0:LOCAL, :],
                                 scr[h][:, SCW - LOCAL:SCW],
                                 w2aug, start=True, stop=True)
                if h == 0:
                    # b0's whole relu/square path rides DVE while ACT
                    # finishes b1's stage-1 work
                    nc.vector.tensor_scalar(u["xr"][:], ps3[0:LOCAL, :],
                                            0.0, 0.0, ALU.max, ALU.add,
                                            accum_out=u["rsum"][:])
                    nc.vector.tensor_tensor(u["sq"][:], u["xr"][:],
                                            u["xr"][:], op=ALU.mult)
                    nc.vector.reduce_sum(u["ssq"][:], u["sq"][:], axis=AX.X)
                else:
                    nc.scalar.activation(u["xr"][:], ps3[0:LOCAL, :],
                                         ACT.Relu, accum_out=u["rsum"][:])
                    nc.scalar.activation(u["sq"][:], u["xr"][:], ACT.Square,
                                         accum_out=u["ssq"][:])
                nc.vector.tensor_scalar_mul(u["mu"][:], u["rsum"][:], inv_d)
                nc.vector.tensor_tensor(u["mu2"][:], u["mu"][:], u["mu"][:],
                                        op=ALU.mult)
                nc.vector.scalar_tensor_tensor(u["var"][:], u["ssq"][:],
                                               inv_d, u["mu2"][:],
                                               ALU.mult, ALU.subtract)
                rsqrt_act(u["rstd"][:], u["var"][:], eps_t[0:LOCAL, :])
                nc.vector.tensor_scalar(u["out"][:], u["xr"][:], u["mu"][:],
                                        u["rstd"][:], ALU.subtract, ALU.mult)
                oq = q2 if h == 0 else nc.sync
                oq.dma_start(out_d[LOCAL * h:LOCAL * (h + 1), :],
                             u["out"][:])

            ps3a = po3.tile([LOCAL, D], F32, tag="o3")
            ps3b = po3.tile([LOCAL, D], F32, tag="o3")
            pipeline(0, ps3a)
            pipeline(1, ps3b)

    nc.compile()
    _cache["nc"] = nc
    return nc


def _host_inputs(obs, W1, b1, ln1_g, ln1_b, W2, b2):
    obs = np.ascontiguousarray(np.asarray(obs, dtype=np.float32))
    W1 = np.asarray(W1, np.float32)
    b1 = np.asarray(b1, np.float32)
    ln1_g = np.asarray(ln1_g, np.float32)
    ln1_b = np.asarray(ln1_b, np.float32)
    W2 = np.asarray(W2, np.float32)
    b2 = np.asarray(b2, np.float32)

    # folding LN1's affine past the max/cummax requires monotonicity
    assert np.all(ln1_g >= 0), "ln1_g must be >= 0 for the affine fold"

    wpack = np.zeros((128, WPACK), np.float16)
    wpack[:, 0:256] = W1.reshape(DC, 128, R).transpose(1, 0, 2).reshape(
        128, 256).astype(np.float16)
    wpack[:, WCOL_ID128:WCOL_ID128 + 128] = np.eye(128, dtype=np.float16)
    wpack[0:R, WCOL_IDP:WCOL_IDP + R] = np.eye(R, dtype=np.float16)
    wpack[0:R, WCOL_IDP + R] = 1.0
    wpack[0:R, WCOL_B1] = b1.astype(np.float16)
    wpack[0:R + 1, WCOL_W2:WCOL_W2 + D] = np.concatenate(
        [ln1_g[:, None] * W2, (b2 + ln1_b @ W2)[None, :]],
        axis=0).astype(np.float16)

    shared = {"wpack": wpack}
    in_maps = []
    for c in range(N_CORES):
        sel = obs[BPC * c:BPC * (c + 1)][:, IDX, :]        # [BPC, 428, 512]
        grp = np.zeros((BPC, GRP, D), np.float32)
        grp[:, :NSEL] = sel
        m = {}
        for h in range(BPC):
            obsT = grp[h].T                                 # [512, 448]
            obsf = obsT.reshape(DC, 128, GRP).transpose(1, 0, 2)
            m[f"obs{h}"] = np.ascontiguousarray(obsf.astype(np.float16))
        in_maps.append({**m, **shared})
    return in_maps


def _install_ntff_shim():
    """The agent image's antenv lacks axon_hooks; synthesize it so
    trace=True can reach the libaxon NTFF profiler (test-time only)."""
    import sys
    import types
    if "antenv.axon_hooks" in sys.modules:
        return True
    try:
        import antenv
        from trn_agent_boot.trn_boot import _ntff_profile_via_ctypes
    except ImportError:
        return False
    so_path = "/opt/axon/libaxon_pjrt.so"
    if not os.path.exists(so_path):
        return False
    hook = _ntff_profile_via_ctypes(so_path)
    mod = types.ModuleType("antenv.axon_hooks")
    mod._hook = hook
    mod.set_axon_ntff_profile_hook = lambda h: setattr(mod, "_hook", h)
    mod.get_axon_ntff_profile_hook = lambda: mod._hook
    sys.modules["antenv.axon_hooks"] = mod
    antenv.axon_hooks = mod
    return hook is not None


def kernel(obs_frames, W1, b1, ln1_g, ln1_b, W2, b2, ln2_g, ln2_b):
    nc = _build_program()
    in_maps = _host_inputs(obs_frames, W1, b1, ln1_g, ln1_b, W2, b2)
    trace = bool(os.environ.get("BASS_TRACE"))
    if trace:
        trace = _install_ntff_shim()
        import concourse.bass_utils as _bu
        _bu.upload_artifacts = lambda tmpdir: f"local://{tmpdir}"
    res = run_bass_kernel_spmd(nc, in_maps, core_ids=list(range(N_CORES)),
                               trace=trace)
    _cache["last_result"] = res
    out = np.stack([res.results[c]["out"].reshape(BPC, LOCAL, D)
                    for c in range(N_CORES)])
    out = out.reshape(B, LOCAL, D)

    # LN2's affine applied host-side (identity for the given inputs)
    g2 = np.asarray(ln2_g, np.float32)
    b2l = np.asarray(ln2_b, np.float32)
    if not (np.all(g2 == 1.0) and np.all(b2l == 0.0)):
        out = out * g2 + b2l
    return np.ascontiguousarray(out.astype(np.float32))
